# revision 5
# baseline (speedup 1.0000x reference)
"""Trainium2 Bass kernel for nn_CrossAttention (B=4, C=256, H=W=64).

Per (batch, branch) the computation is an independent cross-attention:
    f = Wf @ other + bf          [32, 4096]
    g = Wg @ own   + bg          [32, 4096]
    h = Wh @ own   + bh          [256, 4096]
    S = f^T @ g                  [4096, 4096]
    att = softmax(S, axis=-1)
    sa[c, m] = sum_n h[c, n] * att[n, m]
    out = gamma * sa + own

B*2 = 8 independent problems -> one per NeuronCore (pure SPMD).

Factorization: att[n,m] = E[n,m]/Z[n], E = exp(S - K0), Z = rowsum(E), so
sa = (h^T/Z)^T @ E.  E is computed once per 2048-wide chunk on ACT (Z free
via accum_out), kept SBUF-resident in bf16.

v2 schedule vs the original baseline:
  * f/g conv outputs are replicated across the four 32-partition quadrants
    INSIDE the conv matmuls (column-tiled tile_position=(0,32j)) -- no
    SBUF->SBUF replica DMAs, so the exp pipeline starts ~8us in, not 52us.
  * sa accumulates in a 4-bank PSUM chain over a SUPERGROUP of n-tiles
    (sizes [8,8,8,4,2,2]) with one LDWEIGHTS per hxz tile amortized over
    the 4 m-block matmuls, then a single [128,2048] DVE eviction per pass.
  * conv PSUM shares the same 4-bank ring as the sa passes; conv biases are
    folded into the eviction DVE ops (no bias matmuls).
  * per-supergroup Z (accum cols per tile), chunks ordered h0-first within
    a supergroup so the E ring fits in E_BUFS buffers.
  * last supergroup is 2 tiles; its eviction fuses into the
    out = gamma*(sa_psum + sa_sb) + x epilogue.
"""

import os
import sys
from collections import deque

for _p in ("/opt/trn_rl_repo", "/opt/pypackages"):
    if _p not in sys.path:
        sys.path.insert(0, _p)

os.environ.setdefault("JAX_PLATFORMS", "")

import numpy as np

import concourse.bacc as bacc
import concourse.tile as tile
from concourse import mybir

F32 = mybir.dt.float32
F16 = mybir.dt.float16
BF16 = mybir.dt.bfloat16
AF = mybir.ActivationFunctionType
MULT = mybir.AluOpType.mult
ADD = mybir.AluOpType.add

B, C, H, W = 4, 256, 64, 64
N = H * W            # 4096 pixels
C8 = C // 8          # 32
NT = N // 128        # 32 n-tiles
MB = 512             # PSUM bank of fp32
HALF = 2048          # exp chunk (4 PSUM banks)
ICH = 1024           # input DMA chunk columns
NCH = N // ICH       # 4 chunks per partition-half
K0 = 40.0            # constant subtracted inside exp (cancels in softmax)
E_BUFS = 21          # rotating [128, 2048] bf16 E half-tiles
SG = [(0, 8), (8, 16), (16, 24), (24, 28), (28, 30), (30, 32)]  # tile ranges
SLOT_NS = 2350       # trailing-work budget emitted per exp slot


def build_bass():
    nc = bacc.Bacc()

    own_d = nc.dram_tensor("own16", [C, N], F16, kind="ExternalInput")
    oth_d = nc.dram_tensor("oth16", [C, N], F16, kind="ExternalInput")
    res_d = nc.dram_tensor("own32", [C, N], F32, kind="ExternalInput")
    wf_d = nc.dram_tensor("wf_t", [C, C8], F16, kind="ExternalInput")
    wg_d = nc.dram_tensor("wg_t", [C, C8], F16, kind="ExternalInput")
    wh_d = nc.dram_tensor("wh_t", [C, C], F16, kind="ExternalInput")
    bf_d = nc.dram_tensor("bf_rep", [128, 1], F32, kind="ExternalInput")
    bg_d = nc.dram_tensor("bg_rep", [128, 1], F32, kind="ExternalInput")
    bh_d = nc.dram_tensor("bh2k", [128, HALF], F16, kind="ExternalInput")
    gm_d = nc.dram_tensor("gamma_rep", [128, 1], F32, kind="ExternalInput")
    k0_d = nc.dram_tensor("k0_col", [128, 1], F32, kind="ExternalInput")
    out_d = nc.dram_tensor("out", [C, N], F32, kind="ExternalOutput")

    with tile.TileContext(nc) as tc:
        with (
            tc.tile_pool(name="singles", bufs=1) as singles,
            tc.tile_pool(name="inp", bufs=1) as inp,
            tc.tile_pool(name="epool", bufs=E_BUFS) as epool,
            tc.tile_pool(name="zpool", bufs=3) as zpool,
            tc.tile_pool(name="resp", bufs=3) as resp,
            tc.tile_pool(name="outp", bufs=4) as outp,
            tc.tile_pool(name="ps_s", bufs=1, space="PSUM") as ps_s,
            tc.tile_pool(name="ps_w", bufs=1, space="PSUM") as ps_w,
        ):
            # ---- small constants ----
            wf_sb = [singles.tile([128, C8], F16, name=f"wf{k}") for k in range(2)]
            wg_sb = [singles.tile([128, C8], F16, name=f"wg{k}") for k in range(2)]
            wh_sb = [singles.tile([128, C], F16, name=f"wh{k}") for k in range(2)]
            for k in range(2):
                nc.sync.dma_start(out=wf_sb[k], in_=wf_d[128 * k:128 * (k + 1), :])
                nc.sync.dma_start(out=wg_sb[k], in_=wg_d[128 * k:128 * (k + 1), :])
                nc.sync.dma_start(out=wh_sb[k], in_=wh_d[128 * k:128 * (k + 1), :])
            bf_sb = singles.tile([128, 1], F32)
            bg_sb = singles.tile([128, 1], F32)
            bh_sb = singles.tile([128, HALF], F16)
            gm_sb = singles.tile([128, 1], F32)
            k0_sb = singles.tile([128, 1], F32)
            nc.sync.dma_start(out=bf_sb, in_=bf_d[:, :])
            nc.sync.dma_start(out=bg_sb, in_=bg_d[:, :])
            nc.sync.dma_start(out=bh_sb, in_=bh_d[:, :])
            nc.sync.dma_start(out=gm_sb, in_=gm_d[:, :])
            nc.sync.dma_start(out=k0_sb, in_=k0_d[:, :])

            # chunked inputs own/oth [part-half k][chunk c], critical-path
            # order: own c0,c1 then oth c0,c1 (feeds g-alloc 0 / f-alloc 0).
            own_sb = [[inp.tile([128, ICH], F16, name=f"own{k}_{c}")
                       for c in range(NCH)] for k in range(2)]
            oth_sb = [[inp.tile([128, ICH], F16, name=f"oth{k}_{c}")
                       for c in range(NCH)] for k in range(2)]

            def load_inp(dst, src, c):
                for k in range(2):
                    nc.sync.dma_start(
                        out=dst[k][c],
                        in_=src[128 * k:128 * (k + 1), ICH * c:ICH * (c + 1)])

            for c in (0, 1):
                load_inp(own_sb, own_d, c)
            for c in (0, 1):
                load_inp(oth_sb, oth_d, c)
            for c in (2, 3):
                load_inp(own_sb, own_d, c)
            for c in (2, 3):
                load_inp(oth_sb, oth_d, c)

            # f/g conv outputs, quadrant-replicated: fbig[q] holds m-blocks
            # 4q..4q+3 (columns 512*b), rows = 4 copies of the 32 channels.
            fbig = [singles.tile([128, HALF], F16, name=f"fbig{q}") for q in range(2)]
            gbig = [singles.tile([128, HALF], F16, name=f"gbig{q}") for q in range(2)]
            # hxz[q]: n-tiles 8q..8q+7, [128 n, 256 c] each at cols 256*s
            hxz = [singles.tile([128, HALF], BF16, name=f"hxz{q}") for q in range(4)]
            sa_sb = [singles.tile([128, N], F32, name=f"sa{ch}") for ch in range(2)]

            def conv_fg_alloc(dst, w_sb, src, b_sb, q):
                """Conv 4 m-blocks (2048 cols) of f or g, 4x column-tiled so
                the output lands replicated in all four partition quadrants."""
                ps = ps_w.tile([128, HALF], F32, tag="w", name=f"cfg{q}")
                for b4 in range(4):
                    nb = 4 * q + b4
                    cc, oo = (MB * nb) // ICH, (MB * nb) % ICH
                    for j in range(4):
                        for k in range(2):
                            nc.tensor.matmul(
                                out=ps[32 * j:32 * (j + 1), MB * b4:MB * (b4 + 1)],
                                lhsT=w_sb[k],
                                rhs=src[k][cc][:, oo:oo + MB],
                                start=(k == 0),
                                stop=(k == 1),
                                tile_position=(0, 32 * j),
                            )
                nc.vector.tensor_scalar(
                    out=dst[q], in0=ps, scalar1=b_sb[0:128, 0:1], scalar2=None,
                    op0=ADD)

            def conv_h_alloc(q):
                """Conv 8 n-tiles of h into hxz[q]; bias added by the
                eviction tensor_tensor against the 8x-tiled bh row."""
                ps = ps_w.tile([128, HALF], F32, tag="w", name=f"ch{q}")
                for s in range(8):
                    i = 8 * q + s
                    cc, oo = (128 * i) // ICH, (128 * i) % ICH
                    for k in range(2):
                        nc.tensor.matmul(
                            out=ps[:, C * s:C * (s + 1)],
                            lhsT=own_sb[k][cc][:, oo:oo + 128],
                            rhs=wh_sb[k],
                            start=(k == 0),
                            stop=(k == 1),
                        )
                nc.vector.tensor_add(out=hxz[q], in0=ps, in1=bh_sb)

            e_half = {}

            def stats_chunk(t, h, zp, zcol):
                """S chunk (n-tile t, m half h) -> exp -> E + Z partial."""
                sp = ps_s.tile([128, HALF], F32, tag="s", name=f"s{t}_{h}")
                fo = MB * ((t // 4) % 4) + 128 * (t % 4)
                for j in range(4):
                    nc.tensor.matmul(
                        out=sp[:, MB * j:MB * (j + 1)],
                        lhsT=fbig[t // 16][32 * j:32 * (j + 1), fo:fo + 128],
                        rhs=gbig[h][32 * j:32 * (j + 1), MB * j:MB * (j + 1)],
                        start=True,
                        stop=True,
                        tile_position=(32 * j, 0),
                    )
                et = epool.tile([128, HALF], BF16, name=f"e{t}_{h}", tag="e")
                e_half[(t, h)] = et
                nc.scalar.activation(
                    out=et,
                    in_=sp,
                    func=AF.Exp,
                    bias=k0_sb[:, 0:1],
                    accum_out=zp[:, zcol:zcol + 1],
                )

            def zprep(si, zp):
                """Z = sum of half-sums; hxz *= 1/Z (in place), per tile."""
                a, b = SG[si]
                T = b - a
                zt = zpool.tile([128, 8], F32, tag="zt")
                rz = zpool.tile([128, 8], F32, tag="rz")
                nc.vector.tensor_add(
                    out=zt[:, 0:T], in0=zp[:, 0:2 * T:2], in1=zp[:, 1:2 * T:2])
                nc.vector.reciprocal(out=rz[:, 0:T], in_=zt[:, 0:T])
                for tl in range(T):
                    t = a + tl
                    sl = hxz[t // 8][:, C * (t % 8):C * (t % 8 + 1)]
                    nc.vector.tensor_scalar(
                        out=sl, in0=sl, scalar1=rz[:, tl:tl + 1], scalar2=None,
                        op0=MULT)

            def sa_titer(pw, t, h, ch, first, last):
                """One hxz tile's contribution to pass (h, ch): 4 m-block
                matmuls sharing one stationary hxz slice."""
                lh = hxz[t // 8][:, C * (t % 8) + 128 * ch:C * (t % 8) + 128 * ch + 128]
                et = e_half[(t, h)]
                for j in range(4):
                    nc.tensor.matmul(
                        out=pw[:, MB * j:MB * (j + 1)],
                        lhsT=lh,
                        rhs=et[:, MB * j:MB * (j + 1)],
                        start=first,
                        stop=last,
                    )

            # ---- trailing-work queue ----
            trail = deque()

            def pump(budget):
                while trail and budget > 0:
                    cost, fn = trail.popleft()
                    fn()
                    budget -= cost

            for q in (1,):
                trail.append((2000, lambda q=q: conv_fg_alloc(gbig, wg_sb, own_sb, bg_sb, q)))
                trail.append((2000, lambda q=q: conv_fg_alloc(fbig, wf_sb, oth_sb, bf_sb, q)))
            for q in range(4):
                trail.append((3400, lambda q=q: conv_h_alloc(q)))

            def push_passes(si, final=False):
                """Queue supergroup si's 4 sa passes (and eviction)."""
                a, b = SG[si]
                for h in range(2):
                    for ch in range(2):
                        pw = [None]

                        def open_pass(pw=pw, si=si):
                            pw[0] = ps_w.tile([128, HALF], F32, tag="w",
                                              name=f"sa{si}")

                        for tl in range(b - a):
                            t = a + tl

                            def titer(pw=pw, t=t, h=h, ch=ch,
                                      first=(tl == 0), last=(tl == b - a - 1)):
                                sa_titer(pw[0], t, h, ch, first, last)

                            if tl == 0:
                                trail.append((970, lambda o=open_pass, f=titer: (o(), f())))
                            else:
                                trail.append((970, titer))

                        if not final:
                            def evict(pw=pw, h=h, ch=ch, si=si):
                                dst = sa_sb[ch][:, HALF * h:HALF * (h + 1)]
                                if si == 0:
                                    nc.vector.tensor_copy(out=dst, in_=pw[0])
                                else:
                                    nc.vector.tensor_add(out=dst, in0=dst, in1=pw[0])
                            trail.append((300, evict))
                        else:
                            def fin(pw=pw, h=h, ch=ch):
                                for j in range(4):
                                    ot = outp.tile([128, MB], F32, tag="ot")
                                    nc.vector.scalar_tensor_tensor(
                                        out=ot,
                                        in0=pw[0][:, MB * j:MB * (j + 1)],
                                        scalar=gm_sb[:, 0:1],
                                        in1=sa_sb[ch][:, HALF * h + MB * j:HALF * h + MB * (j + 1)],
                                        op0=MULT,
                                        op1=ADD,
                                    )
                                    nc.sync.dma_start(
                                        out=out_d[128 * ch:128 * (ch + 1),
                                                  HALF * h + MB * j:HALF * h + MB * (j + 1)],
                                        in_=ot,
                                    )
                            trail.append((300, fin))

            def push_restmp():
                """res loads + in-place tmp = gamma*sa_sb + res."""
                for ch in range(2):
                    for mb in range(8):
                        def rt(ch=ch, mb=mb):
                            xr = resp.tile([128, MB], F32, tag="xr")
                            nc.sync.dma_start(
                                out=xr,
                                in_=res_d[128 * ch:128 * (ch + 1),
                                          MB * mb:MB * (mb + 1)])
                            sl = sa_sb[ch][:, MB * mb:MB * (mb + 1)]
                            nc.vector.scalar_tensor_tensor(
                                out=sl, in0=sl, scalar=gm_sb[:, 0:1], in1=xr,
                                op0=MULT, op1=ADD)
                        trail.append((200, rt))

            # ---- upfront convs feeding the first chunks ----
            conv_fg_alloc(gbig, wg_sb, own_sb, bg_sb, 0)
            conv_fg_alloc(fbig, wf_sb, oth_sb, bf_sb, 0)

            # ---- chunk schedule: per supergroup, h0 of all tiles then h1 ----
            chunks = []
            for si, (a, b) in enumerate(SG):
                chunks += [(si, t, 0) for t in range(a, b)]
                chunks += [(si, t, 1) for t in range(a, b)]

            zps = {}
            cur_si = -1
            for (si, t, h) in chunks:
                if si != cur_si:
                    if cur_si >= 0:
                        zp_prev = zps.pop(cur_si)
                        trail.appendleft((200, lambda s=cur_si, z=zp_prev: zprep(s, z)))
                        # re-order: zprep first, then passes
                        push_passes(cur_si, final=False)
                        if cur_si == len(SG) - 2:
                            push_restmp()
                    cur_si = si
                    zps[si] = zpool.tile([128, 16], F32, tag="zp",
                                         name=f"zp{si}")
                pump(SLOT_NS)
                a, _ = SG[si]
                stats_chunk(t, h, zps[si], 2 * (t - a) + h)

            # tail: last supergroup's Z, passes (fused epilogue), drain
            zp_last = zps.pop(cur_si)
            trail.appendleft((200, lambda s=cur_si, z=zp_last: zprep(s, z)))
            push_passes(cur_si, final=True)
            while trail:
                pump(10**9)

    if not nc.is_finalized():
        nc.finalize()
    return nc


_NC_CACHE = None


def _get_nc():
    global _NC_CACHE
    if _NC_CACHE is None:
        _NC_CACHE = build_bass()
    return _NC_CACHE


def make_in_maps(**inputs):
    """Build the 8 per-core input maps (core 2b = x-branch, 2b+1 = y-branch)."""
    f = lambda a: np.ascontiguousarray(np.asarray(a), dtype=np.float32)
    h16 = lambda a: np.ascontiguousarray(np.asarray(a), dtype=np.float16)
    x = f(inputs["x"]).reshape(B, C, N)
    y = f(inputs["y"]).reshape(B, C, N)
    x16, y16 = x.astype(np.float16), y.astype(np.float16)
    Wfx, bfx = h16(inputs["Wfx"]), f(inputs["bfx"])
    Wgx, bgx = h16(inputs["Wgx"]), f(inputs["bgx"])
    Whx, bhx = h16(inputs["Whx"]), f(inputs["bhx"])
    Wfy, bfy = h16(inputs["Wfy"]), f(inputs["bfy"])
    Wgy, bgy = h16(inputs["Wgy"]), f(inputs["bgy"])
    Why, bhy = h16(inputs["Why"]), f(inputs["bhy"])
    gamma = f(inputs["gamma"])

    rep4 = lambda b: np.ascontiguousarray(np.tile(b, 4).reshape(128, 1))
    gam = np.ascontiguousarray(np.broadcast_to(gamma.reshape(1, 1), (128, 1)))
    bh8 = lambda b: np.ascontiguousarray(
        np.broadcast_to(np.tile(b.astype(np.float16), 8)[None, :], (128, HALF)))

    c16 = lambda a: np.ascontiguousarray(a, dtype=np.float16)
    branch = {
        "x": dict(
            wf_t=c16(Wfy.T), wg_t=c16(Wgx.T), wh_t=c16(Whx.T),
            bf_rep=rep4(bfy), bg_rep=rep4(bgx), bh2k=bh8(bhx),
        ),
        "y": dict(
            wf_t=c16(Wfx.T), wg_t=c16(Wgy.T), wh_t=c16(Why.T),
            bf_rep=rep4(bfx), bg_rep=rep4(bgy), bh2k=bh8(bhy),
        ),
    }

    k0_col = np.full((128, 1), -K0, np.float32)
    in_maps = []
    for b in range(B):
        in_maps.append(dict(own16=x16[b], oth16=y16[b], own32=x[b],
                            gamma_rep=gam, k0_col=k0_col, **branch["x"]))
        in_maps.append(dict(own16=y16[b], oth16=x16[b], own32=y[b],
                            gamma_rep=gam, k0_col=k0_col, **branch["y"]))
    return in_maps


def kernel(**inputs):
    from concourse.bass_utils import run_bass_kernel_spmd

    nc = _get_nc()
    in_maps = make_in_maps(**inputs)
    res = run_bass_kernel_spmd(nc, in_maps, list(range(8))).results
    out_x = np.stack([res[2 * b]["out"] for b in range(B)]).reshape(B, C, H, W)
    out_y = np.stack([res[2 * b + 1]["out"] for b in range(B)]).reshape(B, C, H, W)
    return (out_x, out_y)
